# revision 14
# baseline (speedup 1.0000x reference)
"""GCN3-bias rating loss on 8 Trainium2 NeuronCores (Bass/Tile).

Strategy (dst-sharded bipartite SpMM):
  - Users/items row-sharded contiguously across 8 cores (padded to 128-blocks).
  - Each SpMM runs edge-centric: per 128-edge tile, gather source rows with
    dma_gather (bf16, 256B rows), build a scaled one-hot S matrix on DVE
    (S[e, dst_lane] = val[e] * (iota == dst_lane)), and accumulate
    psum[dst,feat] += S^T @ Xg on the TensorEngine.  Per-(block,chunk)
    accumulation goes to an SBUF fp32 accumulator so gathers can be issued
    chunk-major in large calls (int16 gather indices limit source tables to
    <=32767 rows per chunk).
  - Layer boundary: AllGather of the g1 shards (bf16) across the 8 cores.
  - Rating head: batch rows are bucketed host-side by user owner (stage 1:
    local gather + MLP) and item owner (stage 2: dot + bias + SSE) with an
    AllGather of the MLP outputs in between.  ub/ib/avg/ratings fold into a
    host-computed per-row bias.
  - Each core returns [128, 8] fp32 partial sums; the host combines them
    into (loss, loss2).
"""
import os
import sys

import numpy as np

try:
    import concourse.bass as bass  # noqa: F401
except Exception:  # pragma: no cover
    sys.path.insert(0, "/opt/trn_rl_repo")

import ml_dtypes
import concourse.bass as bass
import concourse.bacc as bacc
import concourse.tile as tile
from concourse import mybir
from concourse.masks import make_identity

BF16 = ml_dtypes.bfloat16
NC_N = 8
P = 128
LAMADA = 0.001
CHUNK_CAP = 32512  # max rows addressable by int16 gather indices (padded)
GCALL_TILES = 48   # max tiles (128 rows each) per dma_gather call

LAST_EXEC_NS = None


def _ceil(a, b):
    return -(-a // b)


def _pack_idx16(idx_arr, lane, tile_idx, owner, n_cores, nt):
    """idx16[core, 16g + lane%16, tile*8 + lane//16] = idx."""
    out = np.zeros((n_cores, P, nt * 8), np.int16)
    r = (lane % 16).astype(np.int64)
    c = tile_idx.astype(np.int64) * 8 + lane // 16
    out[owner, r, c] = idx_arr
    for g in range(1, 8):
        out[:, 16 * g:16 * g + 16, :] = out[:, :16, :]
    return out


def _build_stream(dst, src, val, shard_rows, nblocks, chunk_rows, n_chunks):
    """Edge stream sorted by (owner, chunk, block); per-(block,chunk) tile
    count = global max (uniform across cores for SPMD).  Returns per-core
    packed arrays + per-chunk tile counts."""
    owner = dst // shard_rows
    local = dst % shard_rows
    blk = local // P
    dstl = (local % P).astype(np.float32)
    ch = src // chunk_rows
    srcl = (src % chunk_rows).astype(np.int32)

    key = (owner.astype(np.int64) * n_chunks + ch) * nblocks + blk
    order = np.argsort(key, kind="stable")
    sk = key[order]
    cnt = np.bincount(key, minlength=NC_N * n_chunks * nblocks)
    cnt = cnt.reshape(NC_N, n_chunks, nblocks)
    tk = _ceil(cnt, P).max(axis=(0, 2))  # [n_chunks]
    nt = int((tk * nblocks).sum())

    first = np.r_[True, sk[1:] != sk[:-1]]
    gstart = np.flatnonzero(first)
    gsize = np.diff(np.r_[gstart, sk.size])
    pos = np.arange(sk.size) - np.repeat(gstart, gsize)
    tile_in_g = pos // P
    lane = pos % P
    own_s = owner[order]
    ch_s = ch[order]
    blk_s = blk[order]
    tile_base = np.r_[0, np.cumsum(tk * nblocks)][:-1]  # per chunk
    tile_idx = tile_base[ch_s] + blk_s * tk[ch_s] + tile_in_g

    idx16 = _pack_idx16(srcl[order], lane, tile_idx, own_s, NC_N, nt)
    dstl_a = np.zeros((NC_N, P, nt), BF16)
    val_a = np.zeros((NC_N, P, nt), BF16)
    dstl_a[own_s, lane, tile_idx] = dstl[order].astype(BF16)
    val_a[own_s, lane, tile_idx] = val[order].astype(BF16)
    return idx16, dstl_a, val_a, [int(x) for x in tk]


def _build_bucket(owner_key, n_rows_tile):
    """Bucket batch rows by owner core.  Returns (order per core list,
    n_tiles uniform, positions)."""
    order = np.argsort(owner_key, kind="stable")
    counts = np.bincount(owner_key, minlength=NC_N)
    ntile = max(1, int(_ceil(counts.max(), n_rows_tile)))
    starts = np.r_[0, np.cumsum(counts)][:-1]
    pos = np.arange(owner_key.size) - starts[owner_key[order]]
    return order, counts, starts, pos, ntile


def kernel(edge_u, edge_i, edge_val, d_i, d_j, user0, item_i0, ratings,
           avg_rating, eu, ei, add_w, w1, b1, w2, b2, ub, ib,
           _simulate=False):
    global LAST_EXEC_NS
    edge_u = np.asarray(edge_u, np.int64)
    edge_i = np.asarray(edge_i, np.int64)
    edge_val = np.asarray(edge_val, np.float32)
    d_i = np.asarray(d_i, np.float32)
    d_j = np.asarray(d_j, np.float32)
    user0 = np.asarray(user0, np.int64)
    item_i0 = np.asarray(item_i0, np.int64)
    ratings = np.asarray(ratings, np.float32)
    avg = float(np.asarray(avg_rating).ravel()[0])
    eu = np.asarray(eu, np.float32)
    ei = np.asarray(ei, np.float32)
    add_w = np.asarray(add_w, np.float32)
    w1 = np.asarray(w1, np.float32)
    b1 = np.asarray(b1, np.float32)
    w2 = np.asarray(w2, np.float32)
    b2 = np.asarray(b2, np.float32)
    ub = np.asarray(ub, np.float32)
    ib = np.asarray(ib, np.float32)

    U, D = eu.shape
    I = ei.shape[0]
    B = user0.shape[0]
    assert D == 128

    sh_u = _ceil(U, NC_N * P) * P          # rows per core (users)
    sh_i = _ceil(I, NC_N * P) * P
    nbu = sh_u // P
    nbi = sh_i // P
    up = sh_u * NC_N
    ip = sh_i * NC_N
    ncu = _ceil(up, CHUNK_CAP)             # chunks of the user table
    cru = _ceil(up, ncu * P) * P           # chunk rows (users)
    nci = _ceil(ip, CHUNK_CAP)
    cri = _ceil(ip, nci * P) * P

    # ---------------- host prep ----------------
    eu_pad = np.zeros((up, D), BF16)
    eu_pad[:U] = eu.astype(BF16)
    ei_pad = np.zeros((ip, D), BF16)
    ei_pad[:I] = ei.astype(BF16)

    # u-side stream: dst=user, src=item (shared by L1u and L2u)
    u_idx, u_dstl, u_val, u_tk = _build_stream(
        edge_u, edge_i, edge_val, sh_u, nbu, cri, nci)
    # i-side stream: dst=item, src=user
    i_idx, i_dstl, i_val, i_tk = _build_stream(
        edge_i, edge_u, edge_val, sh_i, nbi, cru, ncu)
    ntu = u_idx.shape[2] // 8
    nti = i_idx.shape[2] // 8

    dcols_u = np.zeros((NC_N, P, nbu), np.float32)
    dcols_i = np.zeros((NC_N, P, nbi), np.float32)
    du_pad = np.zeros(up, np.float32)
    du_pad[:U] = d_i
    di_pad = np.zeros(ip, np.float32)
    di_pad[:I] = d_j
    for c in range(NC_N):
        dcols_u[c] = du_pad[c * sh_u:(c + 1) * sh_u].reshape(nbu, P).T
        dcols_i[c] = di_pad[c * sh_i:(c + 1) * sh_i].reshape(nbi, P).T

    aw_rep = np.tile(add_w[None, :], (P, 1)).astype(np.float32)
    iota4 = np.tile(np.arange(P, dtype=np.float32), 4)[None, :].repeat(P, 0)
    iota4 = iota4.astype(BF16)

    w1_b = w1.astype(BF16)                       # [128, 256]
    w2a_b = w2[:128].astype(BF16)                # [128, 128]
    w2b_b = w2[128:].astype(BF16)
    b1_p = b1.reshape(2, 128).T.astype(np.float32)   # [128, 2]
    b2_p = b2.reshape(1, 128).T.astype(np.float32)   # [128, 1]

    # batch stage 1: bucket by user owner
    ou = (user0 // sh_u).astype(np.int64)
    s1_order, s1_cnt, s1_start, s1_pos, nbt = _build_bucket(ou, P)
    s1_idx = np.zeros((NC_N, P, nbt * 8), np.int16)
    lane1 = s1_pos % P
    tile1 = s1_pos // P
    s1_idx_flat = (user0[s1_order] % sh_u).astype(np.int16)
    s1_idx[ou[s1_order], (lane1 % 16), tile1 * 8 + lane1 // 16] = s1_idx_flat
    for g in range(1, 8):
        s1_idx[:, 16 * g:16 * g + 16, :] = s1_idx[:, :16, :]
    # mlp_full row of batch element b
    mlp_row = np.zeros(B, np.int64)
    mlp_row[s1_order] = ou[s1_order] * (nbt * P) + s1_pos

    # batch stage 2: bucket by item owner
    oi = (item_i0 // sh_i).astype(np.int64)
    s2_order, s2_cnt, s2_start, s2_pos, nt2 = _build_bucket(oi, P)
    lane2 = s2_pos % P
    tile2 = s2_pos // P
    own2 = oi[s2_order]
    s2_mlp = _pack_idx16(mlp_row[s2_order].astype(np.int16), lane2, tile2,
                         own2, NC_N, nt2)
    s2_gi = _pack_idx16((item_i0[s2_order] % sh_i).astype(np.int16), lane2,
                        tile2, own2, NC_N, nt2)
    biasr = np.zeros((NC_N, P, nt2), np.float32)
    mask = np.zeros((NC_N, P, nt2), np.float32)
    bvals = (ub[user0[s2_order]] + ib[item_i0[s2_order]] + avg
             - ratings[s2_order]).astype(np.float32)
    biasr[own2, lane2, tile2] = bvals
    mask[own2, lane2, tile2] = 1.0

    # ---------------- bass program ----------------
    nc = bacc.Bacc("TRN2", target_bir_lowering=False, debug=False,
                   enable_asserts=True, num_devices=NC_N)
    f32 = mybir.dt.float32
    bf16 = mybir.dt.bfloat16
    i16 = mybir.dt.int16

    t_eu = nc.dram_tensor("eu_tab", [up, D], bf16, kind="ExternalInput")
    t_ei = nc.dram_tensor("ei_tab", [ip, D], bf16, kind="ExternalInput")
    t_eus = nc.dram_tensor("eu_self", [sh_u, D], bf16, kind="ExternalInput")
    t_eis = nc.dram_tensor("ei_self", [sh_i, D], bf16, kind="ExternalInput")
    t_uidx = nc.dram_tensor("u_idx", [P, ntu * 8], i16, kind="ExternalInput")
    t_udstl = nc.dram_tensor("u_dstl", [P, ntu], bf16, kind="ExternalInput")
    t_uval = nc.dram_tensor("u_val", [P, ntu], bf16, kind="ExternalInput")
    t_iidx = nc.dram_tensor("i_idx", [P, nti * 8], i16, kind="ExternalInput")
    t_idstl = nc.dram_tensor("i_dstl", [P, nti], bf16, kind="ExternalInput")
    t_ival = nc.dram_tensor("i_val", [P, nti], bf16, kind="ExternalInput")
    t_du = nc.dram_tensor("dcol_u", [P, nbu], f32, kind="ExternalInput")
    t_di = nc.dram_tensor("dcol_i", [P, nbi], f32, kind="ExternalInput")
    t_aw = nc.dram_tensor("aw", [P, 3], f32, kind="ExternalInput")
    t_iota = nc.dram_tensor("iota4", [P, 4 * P], bf16, kind="ExternalInput")
    t_w1 = nc.dram_tensor("w1t", [P, 256], bf16, kind="ExternalInput")
    t_w2a = nc.dram_tensor("w2a", [P, P], bf16, kind="ExternalInput")
    t_w2b = nc.dram_tensor("w2b", [P, P], bf16, kind="ExternalInput")
    t_b1 = nc.dram_tensor("b1p", [P, 2], f32, kind="ExternalInput")
    t_b2 = nc.dram_tensor("b2p", [P, 1], f32, kind="ExternalInput")
    t_s1 = nc.dram_tensor("s1_idx", [P, nbt * 8], i16, kind="ExternalInput")
    t_s2m = nc.dram_tensor("s2_mlp", [P, nt2 * 8], i16, kind="ExternalInput")
    t_s2g = nc.dram_tensor("s2_gi", [P, nt2 * 8], i16, kind="ExternalInput")
    t_br = nc.dram_tensor("biasr", [P, nt2], f32, kind="ExternalInput")
    t_mk = nc.dram_tensor("mask", [P, nt2], f32, kind="ExternalInput")
    t_out = nc.dram_tensor("partials", [P, 8], f32, kind="ExternalOutput")

    ngru = _ceil(nbu, 4)
    ngri = _ceil(nbi, 4)

    with tile.TileContext(nc) as tc:
        with tc.tile_pool(name="dram", bufs=1, space="DRAM") as dram, \
             tc.tile_pool(name="const", bufs=1) as cp, \
             tc.tile_pool(name="stream", bufs=1) as sp, \
             tc.tile_pool(name="acc", bufs=1) as accp, \
             tc.tile_pool(name="stage", bufs=5) as stg, \
             tc.tile_pool(name="s4", bufs=6) as s4p, \
             tc.tile_pool(name="ep", bufs=2) as ep, \
             tc.tile_pool(name="ps", bufs=4, space="PSUM") as pp, \
             tc.tile_pool(name="pst", bufs=2, space="PSUM") as ppt, \
             tc.tile_pool(name="ps512", bufs=2, space="PSUM") as pp5:

            g1u_sh = dram.tile([sh_u, D], bf16)
            g1i_sh = dram.tile([sh_i, D], bf16)
            g1u_full = dram.tile([up, D], bf16)
            g1i_full = dram.tile([ip, D], bf16)
            gu_aug = dram.tile([sh_u, D], bf16)
            gi_aug = dram.tile([sh_i, D], bf16)
            mlp_sh = dram.tile([nbt * P, D], bf16)
            mlp_full = dram.tile([NC_N * nbt * P, D], bf16)

            # ---- constants ----
            iota_t = cp.tile([P, 4 * P], bf16)
            nc.sync.dma_start(out=iota_t[:], in_=t_iota[:])
            ident = cp.tile([P, P], bf16)
            make_identity(nc, ident[:])
            du_t = cp.tile([P, nbu], f32)
            nc.sync.dma_start(out=du_t[:], in_=t_du[:])
            di_t = cp.tile([P, nbi], f32)
            nc.sync.dma_start(out=di_t[:], in_=t_di[:])
            aw_t = cp.tile([P, 3], f32)
            nc.sync.dma_start(out=aw_t[:], in_=t_aw[:])
            w1_t = cp.tile([P, 256], bf16)
            nc.sync.dma_start(out=w1_t[:], in_=t_w1[:])
            w2a_t = cp.tile([P, P], bf16)
            nc.sync.dma_start(out=w2a_t[:], in_=t_w2a[:])
            w2b_t = cp.tile([P, P], bf16)
            nc.sync.dma_start(out=w2b_t[:], in_=t_w2b[:])
            b1_t = cp.tile([P, 2], f32)
            nc.sync.dma_start(out=b1_t[:], in_=t_b1[:])
            b2_t = cp.tile([P, 1], f32)
            nc.sync.dma_start(out=b2_t[:], in_=t_b2[:])
            lgu_t = cp.tile([P, ngru], f32)
            lgi_t = cp.tile([P, ngri], f32)
            part_t = cp.tile([P, 8], f32)
            nc.vector.memset(part_t[:], 0.0)


            def spmm(src_tab, t_idx, t_dstl, t_val, tk_list, nblocks,
                     chunk_rows, n_chunks, acc):
                tbase = 0
                started = [False] * nblocks
                for k in range(n_chunks):
                    tk = tk_list[k]
                    if tk == 0:
                        continue
                    clo = k * chunk_rows
                    chi = min(chunk_rows, src_tab.shape[0] - clo)
                    chunk_ap = src_tab[clo:clo + chi, :]
                    bpc = max(1, GCALL_TILES // tk)
                    for b0 in range(0, nblocks, bpc):
                        nb = min(bpc, nblocks - b0)
                        ntl = nb * tk
                        t0 = tbase + b0 * tk
                        tpc = min(GCALL_TILES, bpc * tk)
                        idx_sl = stg.tile([P, tpc * 8], i16, tag="gsl_i")
                        nc.sync.dma_start(
                            out=idx_sl[:, :ntl * 8],
                            in_=t_idx[:, t0 * 8:(t0 + ntl) * 8])
                        dstl_sl = stg.tile([P, tpc], bf16, tag="gsl_d")
                        nc.sync.dma_start(out=dstl_sl[:, :ntl],
                                          in_=t_dstl[:, t0:t0 + ntl])
                        val_sl = stg.tile([P, tpc], bf16, tag="gsl_v")
                        nc.sync.dma_start(out=val_sl[:, :ntl],
                                          in_=t_val[:, t0:t0 + ntl])
                        st = stg.tile([P, tpc, P], bf16, tag="gst")
                        nc.gpsimd.dma_gather(
                            out_ap=st[:, :ntl, :],
                            in_ap=chunk_ap,
                            idxs_ap=idx_sl[:, :ntl * 8],
                            num_idxs=ntl * P, num_idxs_reg=ntl * P,
                            elem_size=P, single_packet=False)
                        s_tiles = []
                        for j0 in range(0, ntl, 4):
                            n4 = min(4, ntl - j0)
                            s4 = s4p.tile([P, 4, P], bf16, tag="s4")
                            nc.vector.tensor_tensor(
                                out=s4[:, :n4, :],
                                in0=iota_t[:, :n4 * P].rearrange(
                                    "p (a b) -> p a b", b=P),
                                in1=dstl_sl[:, j0:j0 + n4]
                                    .to_broadcast([P, n4, P]),
                                op=mybir.AluOpType.is_equal)
                            nc.vector.tensor_tensor(
                                out=s4[:, :n4, :], in0=s4[:, :n4, :],
                                in1=val_sl[:, j0:j0 + n4]
                                    .to_broadcast([P, n4, P]),
                                op=mybir.AluOpType.mult)
                            s_tiles.append(s4)
                        for bi in range(nb):
                            ps = pp.tile([P, P], f32, tag="ps")
                            for j in range(tk):
                                sl = bi * tk + j
                                nc.tensor.matmul(
                                    ps[:],
                                    lhsT=s_tiles[sl // 4][:, sl % 4, :],
                                    rhs=st[:, sl, :],
                                    start=(j == 0), stop=(j == tk - 1))
                            gb = b0 + bi
                            asl = acc[:, gb * P:(gb + 1) * P]
                            if not started[gb]:
                                nc.vector.tensor_copy(out=asl, in_=ps[:])
                                started[gb] = True
                            else:
                                nc.vector.tensor_add(out=asl, in0=asl,
                                                     in1=ps[:])
                    tbase += tk * nblocks
                for gb in range(nblocks):
                    if not started[gb]:
                        nc.vector.memset(acc[:, gb * P:(gb + 1) * P], 0.0)

            def epilogue_l1(acc, self_tab, dcol_t, nblocks, out_sh):
                for g0 in range(0, nblocks, 4):
                    n4 = min(4, nblocks - g0)
                    prev = ep.tile([P, 4, P], bf16, tag="ep_prev")
                    nc.sync.dma_start(
                        out=prev[:, :n4, :],
                        in_=self_tab[g0 * P:(g0 + n4) * P, :].rearrange(
                            "(a p) c -> p a c", p=P))
                    dterm = ep.tile([P, 4, P], f32, tag="ep_dt")
                    nc.vector.tensor_tensor(
                        out=dterm[:, :n4, :], in0=prev[:, :n4, :],
                        in1=dcol_t[:, g0:g0 + n4].to_broadcast([P, n4, P]),
                        op=mybir.AluOpType.mult)
                    og = ep.tile([P, 4, P], bf16, tag="ep_out")
                    nc.vector.tensor_tensor(
                        out=og[:, :n4, :],
                        in0=acc[:, g0 * P:(g0 + n4) * P].rearrange(
                            "p (a b) -> p a b", b=P),
                        in1=dterm[:, :n4, :], op=mybir.AluOpType.add)
                    nc.vector.tensor_scalar_max(out=og[:, :n4, :],
                                                in0=og[:, :n4, :],
                                                scalar1=0.0)
                    nc.sync.dma_start(
                        out=out_sh[g0 * P:(g0 + n4) * P, :].rearrange(
                            "(a p) c -> p a c", p=P),
                        in_=og[:, :n4, :])

            def epilogue_l2(acc, g1_sh, e_self, dcol_t, nblocks, out_aug,
                            lg_t):
                for gi, g0 in enumerate(range(0, nblocks, 4)):
                    n4 = min(4, nblocks - g0)
                    prev = ep.tile([P, 4, P], bf16, tag="ep_prev")
                    nc.sync.dma_start(
                        out=prev[:, :n4, :],
                        in_=g1_sh[g0 * P:(g0 + n4) * P, :].rearrange(
                            "(a p) c -> p a c", p=P))
                    e0 = ep.tile([P, 4, P], bf16, tag="ep_e0")
                    nc.sync.dma_start(
                        out=e0[:, :n4, :],
                        in_=e_self[g0 * P:(g0 + n4) * P, :].rearrange(
                            "(a p) c -> p a c", p=P))
                    dterm = ep.tile([P, 4, P], f32, tag="ep_dt")
                    nc.vector.tensor_tensor(
                        out=dterm[:, :n4, :], in0=prev[:, :n4, :],
                        in1=dcol_t[:, g0:g0 + n4].to_broadcast([P, n4, P]),
                        op=mybir.AluOpType.mult)
                    g2 = ep.tile([P, 4, P], bf16, tag="ep_g2")
                    nc.vector.tensor_tensor(
                        out=g2[:, :n4, :],
                        in0=acc[:, g0 * P:(g0 + n4) * P].rearrange(
                            "p (a b) -> p a b", b=P),
                        in1=dterm[:, :n4, :], op=mybir.AluOpType.add)
                    nc.vector.tensor_scalar_max(out=g2[:, :n4, :],
                                                in0=g2[:, :n4, :],
                                                scalar1=0.0)
                    # gu = a0*e0 + a1*prev + a2*g2
                    ta = ep.tile([P, 4, P], f32, tag="ep_ta")
                    nc.vector.tensor_scalar(
                        out=ta[:, :n4, :], in0=e0[:, :n4, :],
                        scalar1=aw_t[:, 0:1], scalar2=None,
                        op0=mybir.AluOpType.mult)
                    tb = ep.tile([P, 4, P], f32, tag="ep_tb")
                    nc.vector.tensor_scalar(
                        out=tb[:, :n4, :], in0=prev[:, :n4, :],
                        scalar1=aw_t[:, 1:2], scalar2=None,
                        op0=mybir.AluOpType.mult)
                    nc.vector.tensor_add(out=ta[:, :n4, :],
                                         in0=ta[:, :n4, :],
                                         in1=tb[:, :n4, :])
                    nc.vector.tensor_scalar(
                        out=tb[:, :n4, :], in0=g2[:, :n4, :],
                        scalar1=aw_t[:, 2:3], scalar2=None,
                        op0=mybir.AluOpType.mult)
                    gu4 = ep.tile([P, 4, P], bf16, tag="ep_gu")
                    nc.vector.tensor_add(out=gu4[:, :n4, :],
                                         in0=ta[:, :n4, :],
                                         in1=tb[:, :n4, :])
                    nc.sync.dma_start(
                        out=out_aug[g0 * P:(g0 + n4) * P, :].rearrange(
                            "(a p) c -> p a c", p=P),
                        in_=gu4[:, :n4, :])
                    scr = ep.tile([P, 4, P], f32, tag="ep_scr")
                    nc.scalar.activation(
                        out=scr[:, :n4, :], in_=gu4[:, :n4, :],
                        func=mybir.ActivationFunctionType.Square,
                        accum_out=lg_t[:, gi:gi + 1])

            rg = [list(range(NC_N))]
            byp = mybir.AluOpType.bypass

            # ---- layer 1, item side ----
            acc_i = accp.tile([P, nbi * P], f32, tag="acc")
            spmm(t_eu, t_iidx, t_idstl, t_ival, i_tk, nbi, cru, ncu, acc_i)
            epilogue_l1(acc_i, t_eis, di_t, nbi, g1i_sh)
            nc.gpsimd.collective_compute(
                "AllGather", byp, replica_groups=rg,
                ins=[g1i_sh.opt()], outs=[g1i_full.opt()])

            # ---- layer 1, user side ----
            acc_u = accp.tile([P, nbu * P], f32, tag="acc")
            spmm(t_ei, t_uidx, t_udstl, t_uval, u_tk, nbu, cri, nci, acc_u)
            epilogue_l1(acc_u, t_eus, du_t, nbu, g1u_sh)
            nc.gpsimd.collective_compute(
                "AllGather", byp, replica_groups=rg,
                ins=[g1u_sh.opt()], outs=[g1u_full.opt()])

            # ---- layer 2, user side (+ gu fuse) ----
            acc_u2 = accp.tile([P, nbu * P], f32, tag="acc")
            spmm(g1i_full, t_uidx, t_udstl, t_uval, u_tk, nbu, cri, nci,
                 acc_u2)
            epilogue_l2(acc_u2, g1u_sh, t_eus, du_t, nbu, gu_aug, lgu_t)

            # ---- batch stage 1: local users -> MLP ----
            s1i_t = sp.tile([P, nbt * 8], i16)
            nc.sync.dma_start(out=s1i_t[:], in_=t_s1[:])
            st1 = cp.tile([P, nbt, P], bf16)
            nc.gpsimd.dma_gather(
                out_ap=st1[:], in_ap=gu_aug[:], idxs_ap=s1i_t[:],
                num_idxs=nbt * P, num_idxs_reg=nbt * P, elem_size=P,
                single_packet=False)
            xt = cp.tile([P, nbt * P], bf16)
            for t in range(nbt):
                pst = ppt.tile([P, P], bf16, tag="pst")
                nc.tensor.transpose(pst[:], st1[:, t, :], ident[:])
                nc.vector.tensor_copy(out=xt[:, t * P:(t + 1) * P],
                                      in_=pst[:])
            h1a = cp.tile([P, nbt * P], bf16)
            h1b = cp.tile([P, nbt * P], bf16)
            ut = cp.tile([P, nbt * P], bf16)
            nb_tot = nbt * P
            for c0 in range(0, nb_tot, 512):
                cw = min(512, nb_tot - c0)
                psa = pp5.tile([P, 512], f32, tag="ps5")
                nc.tensor.matmul(psa[:, :cw], lhsT=w1_t[:, 0:P],
                                 rhs=xt[:, c0:c0 + cw], start=True,
                                 stop=True)
                nc.scalar.activation(
                    out=h1a[:, c0:c0 + cw], in_=psa[:, :cw],
                    func=mybir.ActivationFunctionType.Lrelu,
                    bias=b1_t[:, 0:1], alpha=0.1)
                psb = pp5.tile([P, 512], f32, tag="ps5")
                nc.tensor.matmul(psb[:, :cw], lhsT=w1_t[:, P:2 * P],
                                 rhs=xt[:, c0:c0 + cw], start=True,
                                 stop=True)
                nc.scalar.activation(
                    out=h1b[:, c0:c0 + cw], in_=psb[:, :cw],
                    func=mybir.ActivationFunctionType.Lrelu,
                    bias=b1_t[:, 1:2], alpha=0.1)
                psc = pp5.tile([P, 512], f32, tag="ps5")
                nc.tensor.matmul(psc[:, :cw], lhsT=w2a_t[:],
                                 rhs=h1a[:, c0:c0 + cw], start=True,
                                 stop=False)
                nc.tensor.matmul(psc[:, :cw], lhsT=w2b_t[:],
                                 rhs=h1b[:, c0:c0 + cw], start=False,
                                 stop=True)
                nc.scalar.activation(
                    out=ut[:, c0:c0 + cw], in_=psc[:, :cw],
                    func=mybir.ActivationFunctionType.Lrelu,
                    bias=b2_t[:, 0:1], alpha=0.1)
            urows = cp.tile([P, nbt, P], bf16)
            for t in range(nbt):
                pst = ppt.tile([P, P], bf16, tag="pst")
                nc.tensor.transpose(pst[:], ut[:, t * P:(t + 1) * P],
                                    ident[:])
                nc.vector.tensor_copy(out=urows[:, t, :], in_=pst[:])
            nc.sync.dma_start(
                out=mlp_sh[:].rearrange("(a p) c -> p a c", p=P),
                in_=urows[:])
            nc.gpsimd.collective_compute(
                "AllGather", byp, replica_groups=rg,
                ins=[mlp_sh.opt()], outs=[mlp_full.opt()])

            # ---- layer 2, item side (+ gi fuse) ----
            acc_i2 = accp.tile([P, nbi * P], f32, tag="acc")
            spmm(g1u_full, t_iidx, t_idstl, t_ival, i_tk, nbi, cru, ncu,
                 acc_i2)
            epilogue_l2(acc_i2, g1i_sh, t_eis, di_t, nbi, gi_aug, lgi_t)

            # ---- batch stage 2 ----
            s2m_t = sp.tile([P, nt2 * 8], i16)
            nc.sync.dma_start(out=s2m_t[:], in_=t_s2m[:])
            s2g_t = sp.tile([P, nt2 * 8], i16)
            nc.sync.dma_start(out=s2g_t[:], in_=t_s2g[:])
            br_t = sp.tile([P, nt2], f32)
            nc.sync.dma_start(out=br_t[:], in_=t_br[:])
            mk_t = sp.tile([P, nt2], f32)
            nc.sync.dma_start(out=mk_t[:], in_=t_mk[:])
            stm = cp.tile([P, nt2, P], bf16)
            nc.gpsimd.dma_gather(
                out_ap=stm[:], in_ap=mlp_full[:], idxs_ap=s2m_t[:],
                num_idxs=nt2 * P, num_idxs_reg=nt2 * P, elem_size=P,
                single_packet=False)
            stg2 = cp.tile([P, nt2, P], bf16)
            nc.gpsimd.dma_gather(
                out_ap=stg2[:], in_ap=gi_aug[:], idxs_ap=s2g_t[:],
                num_idxs=nt2 * P, num_idxs_reg=nt2 * P, elem_size=P,
                single_packet=False)
            prod = cp.tile([P, nt2, P], f32)
            nc.vector.tensor_tensor(out=prod[:], in0=stm[:], in1=stg2[:],
                                    op=mybir.AluOpType.mult)
            pred = cp.tile([P, nt2], f32)
            nc.vector.reduce_sum(out=pred[:], in_=prod[:],
                                 axis=mybir.AxisListType.X)
            nc.vector.tensor_add(out=pred[:], in0=pred[:], in1=br_t[:])
            nc.vector.tensor_tensor(out=pred[:], in0=pred[:], in1=mk_t[:],
                                    op=mybir.AluOpType.mult)
            scr2 = cp.tile([P, nt2], f32)
            nc.scalar.activation(
                out=scr2[:], in_=pred[:],
                func=mybir.ActivationFunctionType.Square,
                accum_out=part_t[:, 0:1])
            nc.vector.reduce_sum(out=part_t[:, 1:2], in_=lgu_t[:],
                                 axis=mybir.AxisListType.X)
            nc.vector.reduce_sum(out=part_t[:, 2:3], in_=lgi_t[:],
                                 axis=mybir.AxisListType.X)
            nc.sync.dma_start(out=t_out[:], in_=part_t[:])

    nc.compile()

    # ---------------- run ----------------
    in_maps = []
    for c in range(NC_N):
        in_maps.append({
            "eu_tab": eu_pad, "ei_tab": ei_pad,
            "eu_self": eu_pad[c * sh_u:(c + 1) * sh_u],
            "ei_self": ei_pad[c * sh_i:(c + 1) * sh_i],
            "u_idx": u_idx[c], "u_dstl": u_dstl[c], "u_val": u_val[c],
            "i_idx": i_idx[c], "i_dstl": i_dstl[c], "i_val": i_val[c],
            "dcol_u": dcols_u[c], "dcol_i": dcols_i[c],
            "aw": aw_rep, "iota4": iota4,
            "w1t": w1_b, "w2a": w2a_b, "w2b": w2b_b,
            "b1p": b1_p, "b2p": b2_p,
            "s1_idx": s1_idx[c], "s2_mlp": s2_mlp[c], "s2_gi": s2_gi[c],
            "biasr": biasr[c], "mask": mask[c],
        })

    if _simulate:
        from concourse.bass_interp import MultiCoreSim
        sim = MultiCoreSim(nc, num_cores=NC_N, trace=False,
                           require_finite=False, require_nnan=False)
        cores = list(sim.cores.values())
        for c, core in enumerate(cores):
            for k, v in in_maps[c].items():
                core.tensor(k)[:] = v
        sim.simulate(check_with_hw=False)
        parts = [np.asarray(core.tensor("partials"), np.float64)
                 for core in cores]
    else:
        from concourse.bass_utils import run_bass_kernel_spmd
        trace = os.environ.get("KBENCH_TRACE") == "1"
        res = run_bass_kernel_spmd(nc, in_maps, core_ids=list(range(NC_N)),
                                   trace=trace)
        LAST_EXEC_NS = res.exec_time_ns
        parts = [np.asarray(res.results[c]["partials"], np.float64)
                 for c in range(NC_N)]

    sse = sum(p[:, 0].sum() for p in parts)
    sgu = sum(p[:, 1].sum() for p in parts)
    sgi = sum(p[:, 2].sum() for p in parts)
    loss2 = sse / B
    l2 = LAMADA * (sgu / (U * D)) + LAMADA * (sgi / (I * D))
    loss = loss2 + l2
    return np.float32(loss), np.float32(loss2)
